# revision 34
# baseline (speedup 1.0000x reference)
"""Chunked local attention with global landmarks — Trainium2 Bass kernel.

Full (unsharded) inputs in, full output out. Internally shards across 8
NeuronCores: core i handles chunks [2i, 2i+1] of each batch (4 (b,chunk)
pairs = 2048 query tokens per core). Landmark means are computed per-core
(each 256-token segment lies inside exactly one 512-token chunk) and
replicated with a small AllGather.

v2 layout strategy (all-bf16 dataflow, fp32 PSUM accumulation):
  - x arrives bf16 feature-major [768, 2048]; weights arrive bf16 from the
    host. No on-device fp32r rounding pass at all.
  - QT/KT computed feature-major [o, t]; V token-major [t, h, 65] with a
    parity-dependent ones column (even head: ones in col 64 -> PV psum rows
    [0..64] = [feats, sums]; odd head: ones in col 0 -> PV psum rows
    [63..127] = [sums, feats]).  The odd-head PV thus lands its features at
    partitions 64..127 directly — no partition-shift DMA.
  - scores transposed [k, q]; exp is one ACT per [128,1024] psum with the
    hd^-0.5 fold into the activation scale (|scaled| < 7, no max pass).
  - softmax sums: each head's sum row is DMA-gathered into srow[12, 512];
    one reciprocal_approx_fast per pair + a tiny [12->128] selection matmul
    broadcasts 1/sum across partitions; 6 DVE muls normalize into aoT.
  - output projection computed transposed (yT [o, t]): stationary = Wo
    tiles, moving = aoT; bias lands per-partition on the Scalar engine.
    Host transposes yT back when assembling.
  - emission order keeps the PE stream dense (HAM clock stays at 2.4 GHz):
    weights+x first, collective triggered ~14us in, lm-dependent matmuls
    emitted after two pairs of projections, O(p) after proj(p+1).
"""

import os

import numpy as np

D = 768
H = 12
HD = 64
CH = 512
NLM = 32
B = 2
S = 8192
NCORES = 8
NCHUNK = S // CH           # 16
CPC = NCHUNK // NCORES     # 2 chunks per core per batch
NPAIR = B * CPC            # 4 (batch, chunk) pairs per core
TOK = NPAIR * CH           # 2048 tokens per core
JD = D // 128              # 6 feature tiles
SEG = S // NLM             # 256 tokens per landmark segment
SCALE = float(HD) ** -0.5
NKT = 4                    # local key tiles of 128 (512 = 4*128)

_CACHE = {}


def _build():
    """Build the SPMD Bass/Tile program (same program on all 8 cores)."""
    from contextlib import ExitStack

    import concourse.bass as bass
    import concourse.tile as tile
    from concourse import bacc, mybir

    f32 = mybir.dt.float32
    bf16 = mybir.dt.bfloat16
    Ident = mybir.ActivationFunctionType.Identity
    Exp = mybir.ActivationFunctionType.Exp

    nc = bacc.Bacc(
        "TRN2",
        target_bir_lowering=False,
        debug=False,
        num_devices=NCORES,
    )

    xT_d = nc.dram_tensor("xT", [128, NPAIR, JD, CH], bf16, kind="ExternalInput").ap()
    wqT_d = nc.dram_tensor("wqT", [128, JD, D], bf16, kind="ExternalInput").ap()
    wkT_d = nc.dram_tensor("wkT", [128, JD, D], bf16, kind="ExternalInput").ap()
    wvT_d = nc.dram_tensor("wvT", [128, JD, D], bf16, kind="ExternalInput").ap()
    woT_d = nc.dram_tensor("woT", [128, JD, D], bf16, kind="ExternalInput").ap()
    bq_d = nc.dram_tensor("bq", [D], f32, kind="ExternalInput").ap()
    bk_d = nc.dram_tensor("bk", [D], f32, kind="ExternalInput").ap()
    bv_d = nc.dram_tensor("bv", [D], f32, kind="ExternalInput").ap()
    bo_d = nc.dram_tensor("bo", [D], f32, kind="ExternalInput").ap()
    sel_d = nc.dram_tensor("sel", [H, D], bf16, kind="ExternalInput").ap()
    yT_d = nc.dram_tensor("yT", [128, NPAIR, JD, CH], f32, kind="ExternalOutput").ap()

    # landmark partial sums: [1, 128, JD, 8] -> allgather -> [8, 128, JD, 8]
    lm_part_d = nc.dram_tensor("lm_part", [1, 128, JD, 2 * NPAIR], f32).ap()
    lm_all_d = nc.dram_tensor(
        "lm_all", [NCORES, 128, JD, 2 * NPAIR], f32, addr_space="Shared"
    ).ap()

    with tile.TileContext(nc) as tc, ExitStack() as ctx:
        wpool = ctx.enter_context(tc.tile_pool(name="w", bufs=1))
        const = ctx.enter_context(tc.tile_pool(name="const", bufs=1))
        xpool = ctx.enter_context(tc.tile_pool(name="x", bufs=1))
        qkv = ctx.enter_context(tc.tile_pool(name="qkv", bufs=3))
        ppool = ctx.enter_context(tc.tile_pool(name="p", bufs=3))
        aopool = ctx.enter_context(tc.tile_pool(name="ao", bufs=2))
        aeopool = ctx.enter_context(tc.tile_pool(name="aeo", bufs=1))
        ypool = ctx.enter_context(tc.tile_pool(name="y", bufs=2))
        small = ctx.enter_context(tc.tile_pool(name="small", bufs=1))
        # PSUM: 2 wide slots (2 banks each) + 4 narrow slots (1 bank) = 8 banks
        psW = ctx.enter_context(tc.tile_pool(name="psW", bufs=2, space="PSUM"))
        psN = ctx.enter_context(tc.tile_pool(name="psN", bufs=4, space="PSUM"))

        # ---- resident const tiles ----
        bq_s = const.tile([128, JD], f32, tag="bq")
        bk_s = const.tile([128, JD], f32, tag="bk")
        bo_s = const.tile([128, JD], f32, tag="bo")
        for b_s, b_d in ((bq_s, bq_d), (bk_s, bk_d), (bo_s, bo_d)):
            nc.sync.dma_start(out=b_s[:], in_=b_d.rearrange("(j p) -> p j", p=128))
        bv_bc = const.tile([128, D], f32, tag="bv_bc")
        src = bass.AP(tensor=bv_d.tensor, offset=bv_d.offset, ap=[[0, 128]] + list(bv_d.ap))
        nc.sync.dma_start(out=bv_bc[:], in_=src)
        sel_s = const.tile([H, D], bf16, tag="sel")
        nc.sync.dma_start(out=sel_s[:], in_=sel_d)

        # ---- x loads (sync queue) + weight loads (scalar queue, parallel) ----
        xt = [xpool.tile([128, JD, CH], bf16, tag=f"xt{p}", name=f"xt{p}")
              for p in range(NPAIR)]
        wq_s = wpool.tile([128, JD, D], bf16, tag="wq")
        wk_s = wpool.tile([128, JD, D], bf16, tag="wk")
        wv_s = wpool.tile([128, JD, D], bf16, tag="wv")
        wo_s = wpool.tile([128, JD, D], bf16, tag="wo")
        for w_s, w_d in ((wq_s, wqT_d), (wk_s, wkT_d), (wv_s, wvT_d), (wo_s, woT_d)):
            nc.scalar.dma_start(out=w_s[:], in_=w_d)
        lm_ps = const.tile([128, JD, 2 * NPAIR], f32, tag="lm_ps")
        for p in range(NPAIR):
            nc.sync.dma_start(out=xt[p][:], in_=xT_d[:, p, :, :])
            b, _ = divmod(p, CPC)
            off = b * 4 + 2 * (p % CPC)
            for j in range(JD):
                nc.vector.reduce_sum(
                    out=lm_ps[:, j, off : off + 2],
                    in_=xt[p][:, j, :].rearrange("p (s t) -> p s t", t=SEG),
                    axis=mybir.AxisListType.X,
                )
        nc.gpsimd.dma_start(out=lm_part_d[0], in_=lm_ps[:])
        nc.gpsimd.collective_compute(
            "AllGather",
            mybir.AluOpType.bypass,
            replica_groups=[list(range(NCORES))],
            ins=[lm_part_d[:]],
            outs=[lm_all_d[:]],
        )

        # landmark readback + scale (gpsimd queue: keeps Vector/Scalar clean)
        lmraw_s = const.tile([128, JD, B * NLM], f32, tag="lmraw")
        lmT_s = const.tile([128, JD, B * NLM], bf16, tag="lmT")
        nc.gpsimd.dma_start(
            out=lmraw_s[:].rearrange("p j (b c s) -> p j b c s", c=NCORES, s=4),
            in_=lm_all_d.rearrange("c p j (b s) -> p j b c s", s=4),
        )
        nc.gpsimd.tensor_scalar_mul(lmT_s[:], lmraw_s[:], 1.0 / SEG)

        klmT_s = const.tile([128, JD, B * NLM], bf16, tag="klmT")
        # per-batch landmark V.  Even heads: [64 feats, ones] -> PV rows
        # [0..64] = [feats, sums].  Odd heads: [ones, 63 zeros, 64 feats] ->
        # PV rows [0..128] = [sums, 0.., feats@64-127] (output base must be 0).
        vlme_s = [
            const.tile([NLM, H // 2, HD + 1], bf16, tag=f"vlme{b}", name=f"vlme{b}")
            for b in range(B)
        ]
        vlmo_s = [
            const.tile([NLM, H // 2, 128], bf16, tag=f"vlmo{b}", name=f"vlmo{b}")
            for b in range(B)
        ]

        def emit_lm_kv():
            # landmark K^T: [o, tok] feature-major, both batches at once
            for jo in range(JD):
                ps = psN.tile([128, CH], f32, tag="ps_n")
                for jd in range(JD):
                    nc.tensor.matmul(
                        ps[:, : B * NLM],
                        wk_s[:, jd, jo * 128 : (jo + 1) * 128],
                        lmT_s[:, jd, :],
                        start=(jd == 0),
                        stop=(jd == JD - 1),
                    )
                nc.scalar.activation(
                    out=klmT_s[:, jo, :],
                    in_=ps[:, : B * NLM],
                    func=Ident,
                    bias=bk_s[:, jo : jo + 1],
                    scale=1.0,
                )
            # landmark V: token-major per batch; bias-add + ones on gpsimd
            for b in range(B):
                pw = psW.tile([128, 2 * CH], f32, tag="ps_w")
                for jd in range(JD):
                    lhsT = lmT_s[:, jd, b * NLM : (b + 1) * NLM]
                    nc.tensor.matmul(
                        pw[:NLM, 0:CH], lhsT, wv_s[:, jd, 0:CH],
                        start=(jd == 0), stop=(jd == JD - 1),
                    )
                    nc.tensor.matmul(
                        pw[:NLM, CH:D], lhsT, wv_s[:, jd, CH:D],
                        start=(jd == 0), stop=(jd == JD - 1),
                    )
                pwh = pw[:NLM, 0:D].rearrange("p (a b d) -> p a b d", b=2, d=HD)
                bvh = bv_bc[:NLM, :].rearrange("p (a b d) -> p a b d", b=2, d=HD)
                nc.vector.tensor_add(
                    vlme_s[b][:, :, 0:HD], pwh[:, :, 0, :], bvh[:, :, 0, :]
                )
                nc.vector.tensor_add(
                    vlmo_s[b][:, :, HD:128], pwh[:, :, 1, :], bvh[:, :, 1, :]
                )
                nc.gpsimd.memset(vlme_s[b][:, :, HD : HD + 1], 1.0)
                nc.gpsimd.memset(vlmo_s[b][:, :, 0:1], 1.0)
                nc.gpsimd.memset(vlmo_s[b][:, :, 1:HD], 0.0)

        def emit_proj(p):
            """Q^T/K^T (feature-major) and V (token-major) for pair p."""
            qT = qkv.tile([128, JD, CH], bf16, tag="qT")
            kT = qkv.tile([128, JD, CH], bf16, tag="kT")
            for w_s, out_s, bias_s in ((wq_s, qT, bq_s), (wk_s, kT, bk_s)):
                for jo in range(JD):
                    ps = psN.tile([128, CH], f32, tag="ps_n")
                    for jd in range(JD):
                        nc.tensor.matmul(
                            ps[:],
                            w_s[:, jd, jo * 128 : (jo + 1) * 128],
                            xt[p][:, jd, :],
                            start=(jd == 0),
                            stop=(jd == JD - 1),
                        )
                    nc.scalar.activation(
                        out=out_s[:, jo, :],
                        in_=ps[:],
                        func=Ident,
                        bias=bias_s[:, jo : jo + 1],
                        scale=1.0,
                    )
            v_e = qkv.tile([128, NKT, H // 2, HD + 1], bf16, tag="ve")
            v_o = qkv.tile([128, NKT, H // 2, 128], bf16, tag="vo")
            for tt in range(NKT):
                pw = psW.tile([128, 2 * CH], f32, tag="ps_w")
                for jd in range(JD):
                    lhsT = xt[p][:, jd, tt * 128 : (tt + 1) * 128]
                    nc.tensor.matmul(
                        pw[:, 0:CH], lhsT, wv_s[:, jd, 0:CH],
                        start=(jd == 0), stop=(jd == JD - 1),
                    )
                    nc.tensor.matmul(
                        pw[:, CH:D], lhsT, wv_s[:, jd, CH:D],
                        start=(jd == 0), stop=(jd == JD - 1),
                    )
                pwh = pw[:, 0:D].rearrange("p (a b d) -> p a b d", b=2, d=HD)
                bvh = bv_bc[:, :].rearrange("p (a b d) -> p a b d", b=2, d=HD)
                nc.vector.tensor_add(
                    v_e[:, tt, :, 0:HD], pwh[:, :, 0, :], bvh[:, :, 0, :]
                )
                nc.vector.tensor_add(
                    v_o[:, tt, :, HD:128], pwh[:, :, 1, :], bvh[:, :, 1, :]
                )
            nc.scalar.activation(
                out=v_e[:, :, :, HD : HD + 1],
                in_=bv_bc[:, 0 : NKT * (H // 2)].rearrange(
                    "p (a b c) -> p a b c", a=NKT, b=H // 2
                ),
                func=Ident,
                scale=0.0,
                bias=1.0,
            )
            nc.scalar.activation(
                out=v_o[:, :, :, 0:1],
                in_=bv_bc[:, 0 : NKT * (H // 2)].rearrange(
                    "p (a b c) -> p a b c", a=NKT, b=H // 2
                ),
                func=Ident,
                scale=0.0,
                bias=1.0,
            )
            nc.gpsimd.memset(v_o[:, :, :, 1:HD], 0.0)
            return qT, kT, v_e, v_o

        def emit_att(p, qT, kT, v_e, v_o, pieces=()):
            pieces = list(pieces)
            """Attention for pair p -> (aoE, aoO, srow). Key order: [512 loc, 32 lm].

            aoE[0:64] = even-head features, aoE[64] = even-head sums;
            aoO[64:128] = odd-head features, aoO[63] = odd-head sums.
            """
            b = p // CPC
            aoE = aeopool.tile([HD + 1, JD, CH], bf16, tag="aoE")
            aoO = aeopool.tile([128, JD, CH], bf16, tag="aoO")
            srow = small.tile([H, CH], bf16, tag="srow")
            for h in range(H):
                hp = (h % 2) * 64
                jh = h // 2
                even = h % 2 == 0
                pT = ppool.tile([128, NKT + 1, CH], bf16, tag="pT")
                for g in range(2):
                    ps = psW.tile([128, 2 * CH], f32, tag="ps_w")
                    for i in range(2):
                        kt = 2 * g + i
                        nc.tensor.matmul(
                            ps[:, i * CH : (i + 1) * CH],
                            kT[hp : hp + 64, jh, kt * 128 : (kt + 1) * 128],
                            qT[hp : hp + 64, jh, :],
                            start=True,
                            stop=True,
                        )
                    nc.scalar.activation(
                        out=pT[:, 2 * g : 2 * g + 2, :],
                        in_=ps[:],
                        func=Exp,
                        scale=SCALE,
                    )
                psl = psN.tile([128, CH], f32, tag="ps_n")
                nc.tensor.matmul(
                    psl[:NLM, :],
                    klmT_s[hp : hp + 64, jh, b * NLM : (b + 1) * NLM],
                    qT[hp : hp + 64, jh, :],
                    start=True,
                    stop=True,
                )
                nc.scalar.activation(
                    out=pT[:NLM, NKT, :],
                    in_=psl[:NLM, :],
                    func=Exp,
                    scale=SCALE,
                )

                # PV accumulate; even head -> rows [0:65] = [feats, sums],
                # odd head -> rows [0:128] = [sums, zeros, feats@64-127]
                pv = psN.tile([128, CH], f32, tag="ps_n", name="pv")
                pvs = pv[0 : HD + 1, :] if even else pv[:, :]
                vloc = v_e if even else v_o
                vlm = vlme_s[b] if even else vlmo_s[b]
                for kt in range(NKT):
                    nc.tensor.matmul(
                        pvs,
                        vloc[:, kt, jh, :],
                        pT[:, kt, :],
                        start=(kt == 0),
                        stop=False,
                    )
                nc.tensor.matmul(
                    pvs,
                    vlm[:, jh, :],
                    pT[:NLM, NKT, :],
                    start=False,
                    stop=True,
                )
                # copy features+sums to SBUF (same partitions), then gather
                # the sum row into srow[h] with a small SBUF->SBUF DMA
                if even:
                    nc.vector.tensor_copy(aoE[0 : HD + 1, jh, :], pv[0 : HD + 1, :])
                    nc.sync.dma_start(
                        out=srow[h : h + 1, :], in_=aoE[HD : HD + 1, jh, :]
                    )
                else:
                    nc.vector.tensor_copy(aoO[:, jh, :], pv[:, :])
                    nc.sync.dma_start(
                        out=srow[h : h + 1, :], in_=aoO[0:1, jh, :]
                    )
                if pieces and 2 <= h <= 9:
                    pieces.pop(0)()
            return aoE, aoO, srow

        def emit_norm(p, aoE, aoO, srow):
            """Batched 1/sums + broadcast matmul + normalize into aoT."""
            srowf = small.tile([H, CH], f32, tag="srowf")
            rcf = small.tile([H, CH], f32, tag="rcf")
            rc = small.tile([H, CH], bf16, tag="rc")
            nc.vector.tensor_copy(srowf[:], srow[:])
            nc.vector.reciprocal_approx_fast(out=rcf[:], in_=srowf[:])
            nc.vector.tensor_copy(rc[:], rcf[:])
            aoT = aopool.tile([128, JD, CH], bf16, tag="aoT")
            for jh in range(JD):
                psm = psN.tile([128, CH], f32, tag="ps_n", name="psm")
                nc.tensor.matmul(
                    psm[:],
                    sel_s[:, jh * 128 : (jh + 1) * 128],
                    rc[:],
                    start=True,
                    stop=True,
                )
                nc.vector.tensor_mul(aoT[0:HD, jh, :], aoE[0:HD, jh, :], psm[0:HD, :])
                nc.vector.tensor_mul(
                    aoT[HD:128, jh, :], aoO[HD:128, jh, :], psm[HD:128, :]
                )
            return aoT

        def emit_out_pieces(p, aoT):
            """Output projection as per-jo pieces, transposed: yT[o, t]."""
            yT_s = ypool.tile([128, JD, CH], f32, tag="yT")

            def piece(jo):
                def run():
                    ps = psN.tile([128, CH], f32, tag="ps_n")
                    for jd in range(JD):
                        nc.tensor.matmul(
                            ps[:],
                            wo_s[:, jd, jo * 128 : (jo + 1) * 128],
                            aoT[:, jd, :],
                            start=(jd == 0),
                            stop=(jd == JD - 1),
                        )
                    nc.vector.tensor_scalar_add(
                        yT_s[:, jo, :], ps[:], bo_s[:, jo : jo + 1]
                    )
                return run

            def finalize():
                nc.sync.dma_start(out=yT_d[:, p, :, :], in_=yT_s[:])

            return [piece(jo) for jo in range(JD)], finalize

        def emit_out(p, aoT):
            pieces, fin = emit_out_pieces(p, aoT)
            for pc in pieces:
                pc()
            fin()

        # ---- pipeline: proj0, proj1, lmKV, att0, proj2, N0+O0, att1, ... ----
        qkv0 = emit_proj(0)
        qkv1 = emit_proj(1)
        qkv2 = emit_proj(2)
        emit_lm_kv()
        ao0 = emit_att(0, *qkv0)
        qkv3 = emit_proj(3)
        aoT0 = emit_norm(0, *ao0)
        o0_pieces, o0_fin = emit_out_pieces(0, aoT0)
        ao1 = emit_att(1, *qkv1, pieces=o0_pieces[:5])
        aoT1 = emit_norm(1, *ao1)
        o0_pieces[5]()
        o0_fin()
        o1_pieces, o1_fin = emit_out_pieces(1, aoT1)
        ao2 = emit_att(2, *qkv2, pieces=o1_pieces[:5])
        aoT2 = emit_norm(2, *ao2)
        o1_pieces[5]()
        o1_fin()
        o2_pieces, o2_fin = emit_out_pieces(2, aoT2)
        ao3 = emit_att(3, *qkv3, pieces=o2_pieces[:5])
        aoT3 = emit_norm(3, *ao3)
        o2_pieces[5]()
        o2_fin()
        emit_out(3, aoT3)

    nc.compile()
    return nc


def _shard_inputs(x, Wq, bq, Wk, bk, Wv, bv, Wo, bo):
    import ml_dtypes

    bf = ml_dtypes.bfloat16

    def wtile(W):
        # W [D_out, D_in] -> [128, JD, D_out]: partition-major, contiguous DMA
        return np.ascontiguousarray(
            W.T.reshape(JD, 128, D).transpose(1, 0, 2)
        ).astype(bf)

    wqT = wtile(Wq)
    wkT = wtile(Wk)
    wvT = wtile(Wv)
    woT = wtile(Wo)
    sel = np.zeros((H, D), dtype=bf)
    for h in range(H):
        jh = h // 2
        hp = (h % 2) * 64
        sel[h, jh * 128 + hp : jh * 128 + hp + 64] = 1.0
    in_maps = []
    for c in range(NCORES):
        blocks = []
        for b in range(B):
            for j in range(CPC):
                ch = c * CPC + j
                blocks.append(x[b, ch * CH : (ch + 1) * CH, :])
        xc = np.concatenate(blocks, axis=0)                    # [TOK, D]
        # [128, NPAIR, JD, CH]: xT[p, pair, j, t] = x[pair*CH+t, j*128+p]
        xT = np.ascontiguousarray(
            xc.T.reshape(JD, 128, NPAIR, CH).transpose(1, 2, 0, 3)
        ).astype(bf)
        in_maps.append(
            {
                "xT": xT,
                "wqT": wqT, "wkT": wkT, "wvT": wvT, "woT": woT,
                "bq": np.ascontiguousarray(bq),
                "bk": np.ascontiguousarray(bk),
                "bv": np.ascontiguousarray(bv),
                "bo": np.ascontiguousarray(bo),
                "sel": sel,
            }
        )
    return in_maps


def _assemble(results):
    y = np.empty((B, S, D), dtype=np.float32)
    for c in range(NCORES):
        yt = results[c]["yT"]                                  # [128, NPAIR, JD, CH]
        yc = yt.transpose(1, 3, 2, 0).reshape(TOK, D)          # [TOK, D]
        i = 0
        for b in range(B):
            for j in range(CPC):
                ch = c * CPC + j
                y[b, ch * CH : (ch + 1) * CH, :] = yc[i * CH : (i + 1) * CH, :]
                i += 1
    return y


def kernel(x, Wq, bq, Wk, bk, Wv, bv, Wo, bo):
    from concourse.bass_utils import run_bass_kernel_spmd

    x = np.asarray(x, dtype=np.float32)
    if "nc" not in _CACHE:
        _CACHE["nc"] = _build()
    nc = _CACHE["nc"]
    in_maps = _shard_inputs(
        x,
        np.asarray(Wq), np.asarray(bq),
        np.asarray(Wk), np.asarray(bk),
        np.asarray(Wv), np.asarray(bv),
        np.asarray(Wo), np.asarray(bo),
    )
    trace = bool(int(os.environ.get("KERNEL_TRACE", "0")))
    res = run_bass_kernel_spmd(nc, in_maps, list(range(NCORES)), trace=trace)
    if trace:
        _CACHE["last_exec_time_ns"] = res.exec_time_ns
        _CACHE["last_results"] = res
    return _assemble(res.results)


# revision 38
# speedup vs baseline: 1.1574x; 1.1574x over previous
"""Chunked local attention with global landmarks — Trainium2 Bass kernel.

Full (unsharded) inputs in, full output out. Internally shards across 8
NeuronCores: core i handles chunks [2i, 2i+1] of each batch (4 (b,chunk)
pairs = 2048 query tokens per core). Landmark means are computed per-core
(each 256-token segment lies inside exactly one 512-token chunk) and
replicated with a small AllGather.

v2 layout strategy (all-bf16 dataflow, fp32 PSUM accumulation):
  - x arrives bf16 feature-major [768, 2048]; weights arrive bf16 from the
    host. No on-device fp32r rounding pass at all.
  - QT/KT computed feature-major [o, t]; V token-major [t, h, 65] with a
    parity-dependent ones column (even head: ones in col 64 -> PV psum rows
    [0..64] = [feats, sums]; odd head: ones in col 0 -> PV psum rows
    [63..127] = [sums, feats]).  The odd-head PV thus lands its features at
    partitions 64..127 directly — no partition-shift DMA.
  - scores transposed [k, q]; exp is one ACT per [128,1024] psum with the
    hd^-0.5 fold into the activation scale (|scaled| < 7, no max pass).
  - softmax sums: each head's sum row is DMA-gathered into srow[12, 512];
    one reciprocal_approx_fast per pair + a tiny [12->128] selection matmul
    broadcasts 1/sum across partitions; 6 DVE muls normalize into aoT.
  - output projection computed transposed (yT [o, t]): stationary = Wo
    tiles, moving = aoT; bias lands per-partition on the Scalar engine.
    Host transposes yT back when assembling.
  - emission order keeps the PE stream dense (HAM clock stays at 2.4 GHz):
    weights+x first, collective triggered ~14us in, lm-dependent matmuls
    emitted after two pairs of projections, O(p) after proj(p+1).
"""

import os

import numpy as np

D = 768
H = 12
HD = 64
CH = 512
NLM = 32
B = 2
S = 8192
NCORES = 8
NCHUNK = S // CH           # 16
CPC = NCHUNK // NCORES     # 2 chunks per core per batch
NPAIR = B * CPC            # 4 (batch, chunk) pairs per core
TOK = NPAIR * CH           # 2048 tokens per core
JD = D // 128              # 6 feature tiles
SEG = S // NLM             # 256 tokens per landmark segment
SCALE = float(HD) ** -0.5
NKT = 4                    # local key tiles of 128 (512 = 4*128)

_CACHE = {}


def _build():
    """Build the SPMD Bass/Tile program (same program on all 8 cores)."""
    from contextlib import ExitStack

    import concourse.bass as bass
    import concourse.tile as tile
    from concourse import bacc, mybir

    f32 = mybir.dt.float32
    bf16 = mybir.dt.bfloat16
    Ident = mybir.ActivationFunctionType.Identity
    Exp = mybir.ActivationFunctionType.Exp

    nc = bacc.Bacc(
        "TRN2",
        target_bir_lowering=False,
        debug=False,
        num_devices=NCORES,
    )

    xT_d = nc.dram_tensor("xT", [128, NPAIR, JD, CH], bf16, kind="ExternalInput").ap()
    wqT_d = nc.dram_tensor("wqT", [128, JD, D], bf16, kind="ExternalInput").ap()
    wkT_d = nc.dram_tensor("wkT", [128, JD, D], bf16, kind="ExternalInput").ap()
    wvT_d = nc.dram_tensor("wvT", [128, JD, D], bf16, kind="ExternalInput").ap()
    woT_d = nc.dram_tensor("woT", [128, JD, D], bf16, kind="ExternalInput").ap()
    bq_d = nc.dram_tensor("bq", [D], f32, kind="ExternalInput").ap()
    bk_d = nc.dram_tensor("bk", [D], f32, kind="ExternalInput").ap()
    bv_d = nc.dram_tensor("bv", [128, D], f32, kind="ExternalInput").ap()
    bo_d = nc.dram_tensor("bo", [D], f32, kind="ExternalInput").ap()
    sel_d = nc.dram_tensor("sel", [H, D], bf16, kind="ExternalInput").ap()
    yT_d = nc.dram_tensor("yT", [128, NPAIR, JD, CH], f32, kind="ExternalOutput").ap()

    # landmark partial sums: [1, 128, JD, 8] -> allgather -> [8, 128, JD, 8]
    lm_part_d = nc.dram_tensor("lm_part", [1, 128, JD, 2 * NPAIR], f32).ap()
    lm_all_d = nc.dram_tensor(
        "lm_all", [NCORES, 128, JD, 2 * NPAIR], f32, addr_space="Shared"
    ).ap()

    with tile.TileContext(nc) as tc, ExitStack() as ctx:
        wpool = ctx.enter_context(tc.tile_pool(name="w", bufs=1))
        const = ctx.enter_context(tc.tile_pool(name="const", bufs=1))
        xpool = ctx.enter_context(tc.tile_pool(name="x", bufs=1))
        qkv = ctx.enter_context(tc.tile_pool(name="qkv", bufs=3))
        ppool = ctx.enter_context(tc.tile_pool(name="p", bufs=3))
        aopool = ctx.enter_context(tc.tile_pool(name="ao", bufs=2))
        aeopool = ctx.enter_context(tc.tile_pool(name="aeo", bufs=1))
        ypool = ctx.enter_context(tc.tile_pool(name="y", bufs=1))
        small = ctx.enter_context(tc.tile_pool(name="small", bufs=2))
        # PSUM: 2 wide slots (2 banks each) + 4 narrow slots (1 bank) = 8 banks
        psW = ctx.enter_context(tc.tile_pool(name="psW", bufs=2, space="PSUM"))
        psN = ctx.enter_context(tc.tile_pool(name="psN", bufs=4, space="PSUM"))

        # ---- resident const tiles ----
        bq_s = const.tile([128, JD], f32, tag="bq")
        bk_s = const.tile([128, JD], f32, tag="bk")
        bo_s = const.tile([128, JD], f32, tag="bo")
        for b_s, b_d in ((bq_s, bq_d), (bk_s, bk_d), (bo_s, bo_d)):
            nc.gpsimd.dma_start(out=b_s[:], in_=b_d.rearrange("(j p) -> p j", p=128))
        bv_bc = const.tile([128, D], f32, tag="bv_bc")
        nc.gpsimd.dma_start(out=bv_bc[:], in_=bv_d)
        sel_s = const.tile([H, D], bf16, tag="sel")
        nc.gpsimd.dma_start(out=sel_s[:], in_=sel_d)

        # ---- x loads (sync queue) + weight loads (scalar queue, parallel) ----
        xt = [xpool.tile([128, JD, CH], bf16, tag=f"xt{p}", name=f"xt{p}")
              for p in range(NPAIR)]
        wq_s = wpool.tile([128, JD, D], bf16, tag="wq")
        wk_s = wpool.tile([128, JD, D], bf16, tag="wk")
        wv_s = wpool.tile([128, JD, D], bf16, tag="wv")
        wo_s = wpool.tile([128, JD, D], bf16, tag="wo")
        for w_s, w_d in ((wq_s, wqT_d), (wk_s, wkT_d), (wv_s, wvT_d), (wo_s, woT_d)):
            nc.scalar.dma_start(out=w_s[:], in_=w_d)
        lm_ps = const.tile([128, JD, 2 * NPAIR], f32, tag="lm_ps")
        for p in range(NPAIR):
            nc.sync.dma_start(out=xt[p][:], in_=xT_d[:, p, :, :])
            b, _ = divmod(p, CPC)
            off = b * 4 + 2 * (p % CPC)
            for j in range(JD):
                nc.vector.reduce_sum(
                    out=lm_ps[:, j, off : off + 2],
                    in_=xt[p][:, j, :].rearrange("p (s t) -> p s t", t=SEG),
                    axis=mybir.AxisListType.X,
                )
        nc.gpsimd.dma_start(out=lm_part_d[0], in_=lm_ps[:])
        nc.gpsimd.collective_compute(
            "AllGather",
            mybir.AluOpType.bypass,
            replica_groups=[list(range(NCORES))],
            ins=[lm_part_d[:]],
            outs=[lm_all_d[:]],
        )

        # landmark readback + scale (gpsimd queue: keeps Vector/Scalar clean)
        lmraw_s = const.tile([128, JD, B * NLM], f32, tag="lmraw")
        lmT_s = const.tile([128, JD, B * NLM], bf16, tag="lmT")
        nc.gpsimd.dma_start(
            out=lmraw_s[:].rearrange("p j (b c s) -> p j b c s", c=NCORES, s=4),
            in_=lm_all_d.rearrange("c p j (b s) -> p j b c s", s=4),
        )
        nc.gpsimd.tensor_scalar_mul(lmT_s[:], lmraw_s[:], 1.0 / SEG)

        klmT_s = const.tile([128, JD, B * NLM], bf16, tag="klmT")
        # per-batch landmark V.  Even heads: [64 feats, ones] -> PV rows
        # [0..64] = [feats, sums].  Odd heads: [ones, 63 zeros, 64 feats] ->
        # PV rows [0..128] = [sums, 0.., feats@64-127] (output base must be 0).
        vlme_s = [
            const.tile([NLM, H // 2, HD + 1], bf16, tag=f"vlme{b}", name=f"vlme{b}")
            for b in range(B)
        ]
        vlmo_s = [
            const.tile([NLM, H // 2, 128], bf16, tag=f"vlmo{b}", name=f"vlmo{b}")
            for b in range(B)
        ]

        def emit_lm_kv():
            # landmark K^T: [o, tok] feature-major, both batches at once
            for jo in range(JD):
                ps = psN.tile([128, CH], f32, tag="ps_n")
                for jd in range(JD):
                    nc.tensor.matmul(
                        ps[:, : B * NLM],
                        wk_s[:, jd, jo * 128 : (jo + 1) * 128],
                        lmT_s[:, jd, :],
                        start=(jd == 0),
                        stop=(jd == JD - 1),
                    )
                nc.scalar.activation(
                    out=klmT_s[:, jo, :],
                    in_=ps[:, : B * NLM],
                    func=Ident,
                    bias=bk_s[:, jo : jo + 1],
                    scale=1.0,
                )
            # landmark V: token-major per batch; bias-add + ones on gpsimd
            for b in range(B):
                pw = psW.tile([128, 2 * CH], f32, tag="ps_w")
                for jd in range(JD):
                    lhsT = lmT_s[:, jd, b * NLM : (b + 1) * NLM]
                    nc.tensor.matmul(
                        pw[:NLM, 0:CH], lhsT, wv_s[:, jd, 0:CH],
                        start=(jd == 0), stop=(jd == JD - 1),
                    )
                    nc.tensor.matmul(
                        pw[:NLM, CH:D], lhsT, wv_s[:, jd, CH:D],
                        start=(jd == 0), stop=(jd == JD - 1),
                    )
                pwh = pw[:NLM, 0:D].rearrange("p (a b d) -> p a b d", b=2, d=HD)
                bvh = bv_bc[:NLM, :].rearrange("p (a b d) -> p a b d", b=2, d=HD)
                nc.vector.tensor_add(
                    vlme_s[b][:, :, 0:HD], pwh[:, :, 0, :], bvh[:, :, 0, :]
                )
                nc.vector.tensor_add(
                    vlmo_s[b][:, :, HD:128], pwh[:, :, 1, :], bvh[:, :, 1, :]
                )
                nc.gpsimd.memset(vlme_s[b][:, :, HD : HD + 1], 1.0)
                nc.gpsimd.memset(vlmo_s[b][:, :, 0:1], 1.0)
                nc.gpsimd.memset(vlmo_s[b][:, :, 1:HD], 0.0)

        def emit_proj(p):
            """Q^T/K^T (feature-major) and V (token-major) for pair p."""
            qT = qkv.tile([128, JD, CH], bf16, tag="qT")
            kT = qkv.tile([128, JD, CH], bf16, tag="kT")
            for w_s, out_s, bias_s in ((wq_s, qT, bq_s), (wk_s, kT, bk_s)):
                for jo in range(JD):
                    ps = psN.tile([128, CH], f32, tag="ps_n")
                    for jd in range(JD):
                        nc.tensor.matmul(
                            ps[:],
                            w_s[:, jd, jo * 128 : (jo + 1) * 128],
                            xt[p][:, jd, :],
                            start=(jd == 0),
                            stop=(jd == JD - 1),
                        )
                    nc.scalar.activation(
                        out=out_s[:, jo, :],
                        in_=ps[:],
                        func=Ident,
                        bias=bias_s[:, jo : jo + 1],
                        scale=1.0,
                    )
            v_e = qkv.tile([128, NKT, H // 2, HD + 1], bf16, tag="ve")
            v_o = qkv.tile([128, NKT, H // 2, 128], bf16, tag="vo")
            for tt in range(NKT):
                pw = psW.tile([128, 2 * CH], f32, tag="ps_w")
                for jd in range(JD):
                    lhsT = xt[p][:, jd, tt * 128 : (tt + 1) * 128]
                    nc.tensor.matmul(
                        pw[:, 0:CH], lhsT, wv_s[:, jd, 0:CH],
                        start=(jd == 0), stop=(jd == JD - 1),
                    )
                    nc.tensor.matmul(
                        pw[:, CH:D], lhsT, wv_s[:, jd, CH:D],
                        start=(jd == 0), stop=(jd == JD - 1),
                    )
                pwh = pw[:, 0:D].rearrange("p (a b d) -> p a b d", b=2, d=HD)
                bvh = bv_bc[:, :].rearrange("p (a b d) -> p a b d", b=2, d=HD)
                nc.vector.tensor_add(
                    v_e[:, tt, :, 0:HD], pwh[:, :, 0, :], bvh[:, :, 0, :]
                )
                nc.vector.tensor_add(
                    v_o[:, tt, :, HD:128], pwh[:, :, 1, :], bvh[:, :, 1, :]
                )
            nc.scalar.activation(
                out=v_e[:, :, :, HD : HD + 1],
                in_=bv_bc[:, 0 : NKT * (H // 2)].rearrange(
                    "p (a b c) -> p a b c", a=NKT, b=H // 2
                ),
                func=Ident,
                scale=0.0,
                bias=1.0,
            )
            nc.scalar.activation(
                out=v_o[:, :, :, 0:1],
                in_=bv_bc[:, 0 : NKT * (H // 2)].rearrange(
                    "p (a b c) -> p a b c", a=NKT, b=H // 2
                ),
                func=Ident,
                scale=0.0,
                bias=1.0,
            )
            nc.gpsimd.memset(v_o[:, :, :, 1:HD], 0.0)
            return qT, kT, v_e, v_o

        def emit_att(p, qT, kT, v_e, v_o):
            """Attention for pair p -> (aoE, aoO, srow). Key order: [512 loc, 32 lm].

            aoE[0:64] = even-head features, aoE[64] = even-head sums;
            aoO[64:128] = odd-head features, aoO[63] = odd-head sums.
            """
            b = p // CPC
            aoE = aeopool.tile([HD + 1, JD, CH], bf16, tag="aoE")
            aoO = aeopool.tile([128, JD, CH], bf16, tag="aoO")
            srow = small.tile([H, CH], bf16, tag="srow")
            for h in range(H):
                hp = (h % 2) * 64
                jh = h // 2
                even = h % 2 == 0
                pT = ppool.tile([128, NKT + 1, CH], bf16, tag="pT")
                for g in range(2):
                    ps = psW.tile([128, 2 * CH], f32, tag="ps_w")
                    for i in range(2):
                        kt = 2 * g + i
                        nc.tensor.matmul(
                            ps[:, i * CH : (i + 1) * CH],
                            kT[hp : hp + 64, jh, kt * 128 : (kt + 1) * 128],
                            qT[hp : hp + 64, jh, :],
                            start=True,
                            stop=True,
                        )
                    nc.scalar.activation(
                        out=pT[:, 2 * g : 2 * g + 2, :],
                        in_=ps[:],
                        func=Exp,
                        scale=SCALE,
                    )
                psl = psN.tile([128, CH], f32, tag="ps_n")
                nc.tensor.matmul(
                    psl[:NLM, :],
                    klmT_s[hp : hp + 64, jh, b * NLM : (b + 1) * NLM],
                    qT[hp : hp + 64, jh, :],
                    start=True,
                    stop=True,
                )
                nc.scalar.activation(
                    out=pT[:NLM, NKT, :],
                    in_=psl[:NLM, :],
                    func=Exp,
                    scale=SCALE,
                )

                # PV accumulate; even head -> rows [0:65] = [feats, sums],
                # odd head -> rows [0:128] = [sums, zeros, feats@64-127]
                pv = psN.tile([128, CH], f32, tag="ps_n", name="pv")
                pvs = pv[0 : HD + 1, :] if even else pv[:, :]
                vloc = v_e if even else v_o
                vlm = vlme_s[b] if even else vlmo_s[b]
                for kt in range(NKT):
                    nc.tensor.matmul(
                        pvs,
                        vloc[:, kt, jh, :],
                        pT[:, kt, :],
                        start=(kt == 0),
                        stop=False,
                    )
                nc.tensor.matmul(
                    pvs,
                    vlm[:, jh, :],
                    pT[:NLM, NKT, :],
                    start=False,
                    stop=True,
                )
                # copy features+sums to SBUF (same partitions), then gather
                # the sum row into srow[h] with a small SBUF->SBUF DMA
                if even:
                    nc.vector.tensor_copy(aoE[0 : HD + 1, jh, :], pv[0 : HD + 1, :])
                    nc.sync.dma_start(
                        out=srow[h : h + 1, :], in_=aoE[HD : HD + 1, jh, :]
                    )
                else:
                    nc.vector.tensor_copy(aoO[:, jh, :], pv[:, :])
                    nc.sync.dma_start(
                        out=srow[h : h + 1, :], in_=aoO[0:1, jh, :]
                    )
            return aoE, aoO, srow

        def emit_norm(p, aoE, aoO, srow):
            """Batched 1/sums + broadcast matmul + normalize into aoT."""
            srowf = small.tile([H, CH], f32, tag="srowf")
            rcf = small.tile([H, CH], f32, tag="rcf")
            rc = small.tile([H, CH], bf16, tag="rc")
            nc.vector.tensor_copy(srowf[:], srow[:])
            nc.vector.reciprocal_approx_fast(out=rcf[:], in_=srowf[:])
            nc.vector.tensor_copy(rc[:], rcf[:])
            aoT = aopool.tile([128, JD, CH], bf16, tag="aoT")
            for jh in range(JD):
                psm = psN.tile([128, CH], f32, tag="ps_n", name="psm")
                nc.tensor.matmul(
                    psm[:],
                    sel_s[:, jh * 128 : (jh + 1) * 128],
                    rc[:],
                    start=True,
                    stop=True,
                )
                nc.vector.tensor_mul(aoT[0:HD, jh, :], aoE[0:HD, jh, :], psm[0:HD, :])
                nc.vector.tensor_mul(
                    aoT[HD:128, jh, :], aoO[HD:128, jh, :], psm[HD:128, :]
                )
            return aoT

        def emit_out(p, aoT):
            """Output projection, transposed: yT[o, t] = Wo^T-major."""
            yT_s = ypool.tile([128, JD, CH], f32, tag="yT")
            for jo in range(JD):
                ps = psN.tile([128, CH], f32, tag="ps_n")
                for jd in range(JD):
                    nc.tensor.matmul(
                        ps[:],
                        wo_s[:, jd, jo * 128 : (jo + 1) * 128],
                        aoT[:, jd, :],
                        start=(jd == 0),
                        stop=(jd == JD - 1),
                    )
                nc.vector.tensor_scalar_add(
                    yT_s[:, jo, :], ps[:], bo_s[:, jo : jo + 1]
                )
                nc.sync.dma_start(out=yT_d[:, p, jo, :], in_=yT_s[:, jo, :])

        # ---- pipeline: proj0, proj1, lmKV, att0, proj2, N0+O0, att1, ... ----
        qkv0 = emit_proj(0)
        qkv1 = emit_proj(1)
        qkv2 = emit_proj(2)
        emit_lm_kv()
        ao0 = emit_att(0, *qkv0)
        qkv3 = emit_proj(3)
        emit_out(0, emit_norm(0, *ao0))
        ao1 = emit_att(1, *qkv1)
        emit_out(1, emit_norm(1, *ao1))
        ao2 = emit_att(2, *qkv2)
        emit_out(2, emit_norm(2, *ao2))
        ao3 = emit_att(3, *qkv3)
        emit_out(3, emit_norm(3, *ao3))

    nc.compile()
    return nc


def _shard_inputs(x, Wq, bq, Wk, bk, Wv, bv, Wo, bo):
    import ml_dtypes

    bf = ml_dtypes.bfloat16

    def wtile(W):
        # W [D_out, D_in] -> [128, JD, D_out]: partition-major, contiguous DMA
        return np.ascontiguousarray(
            W.T.reshape(JD, 128, D).transpose(1, 0, 2)
        ).astype(bf)

    wqT = wtile(Wq)
    wkT = wtile(Wk)
    wvT = wtile(Wv)
    woT = wtile(Wo)
    sel = np.zeros((H, D), dtype=bf)
    for h in range(H):
        jh = h // 2
        hp = (h % 2) * 64
        sel[h, jh * 128 + hp : jh * 128 + hp + 64] = 1.0
    in_maps = []
    for c in range(NCORES):
        blocks = []
        for b in range(B):
            for j in range(CPC):
                ch = c * CPC + j
                blocks.append(x[b, ch * CH : (ch + 1) * CH, :])
        xc = np.concatenate(blocks, axis=0)                    # [TOK, D]
        # [128, NPAIR, JD, CH]: xT[p, pair, j, t] = x[pair*CH+t, j*128+p]
        xT = np.ascontiguousarray(
            xc.T.reshape(JD, 128, NPAIR, CH).transpose(1, 2, 0, 3)
        ).astype(bf)
        in_maps.append(
            {
                "xT": xT,
                "wqT": wqT, "wkT": wkT, "wvT": wvT, "woT": woT,
                "bq": np.ascontiguousarray(bq),
                "bk": np.ascontiguousarray(bk),
                "bv": np.ascontiguousarray(np.broadcast_to(bv, (128, D))),
                "bo": np.ascontiguousarray(bo),
                "sel": sel,
            }
        )
    return in_maps


def _assemble(results):
    y = np.empty((B, S, D), dtype=np.float32)
    for c in range(NCORES):
        yt = results[c]["yT"]                                  # [128, NPAIR, JD, CH]
        yc = yt.transpose(1, 3, 2, 0).reshape(TOK, D)          # [TOK, D]
        i = 0
        for b in range(B):
            for j in range(CPC):
                ch = c * CPC + j
                y[b, ch * CH : (ch + 1) * CH, :] = yc[i * CH : (i + 1) * CH, :]
                i += 1
    return y


def kernel(x, Wq, bq, Wk, bk, Wv, bv, Wo, bo):
    from concourse.bass_utils import run_bass_kernel_spmd

    x = np.asarray(x, dtype=np.float32)
    if "nc" not in _CACHE:
        _CACHE["nc"] = _build()
    nc = _CACHE["nc"]
    in_maps = _shard_inputs(
        x,
        np.asarray(Wq), np.asarray(bq),
        np.asarray(Wk), np.asarray(bk),
        np.asarray(Wv), np.asarray(bv),
        np.asarray(Wo), np.asarray(bo),
    )
    trace = bool(int(os.environ.get("KERNEL_TRACE", "0")))
    res = run_bass_kernel_spmd(nc, in_maps, list(range(NCORES)), trace=trace)
    if trace:
        _CACHE["last_exec_time_ns"] = res.exec_time_ns
        _CACHE["last_results"] = res
    return _assemble(res.results)
